# revision 39
# baseline (speedup 1.0000x reference)
"""DendriteLayer Trainium2 kernel.

Math (reference): out0 = x @ (w_in*w_in_mask).T + b_in; a = out0.reshape(B, dpc, out_dim);
winner = argmax_d(a * boost); out1 = a * one_hot(winner); y = out1f @ (w_out*dend_mask).T + b_out.

Sharding: 8 cores, core c owns global units u in [c*256, (c+1)*256) (all dpc=8 dendrites)
and output columns v with (v % 256) in [c*32, (c+1)*32). Both k-winners and the
block-diagonal output stage are then fully local to a core (no collectives).

Per-core j' layout is u'-major interleaved: j' = u'*8 + d, so the 8 dendrites of a
unit are consecutive, and each 512-wide chunk of j' is self-contained for both the
k-winners (max over d) and the output segment-sums.

Matmul: single f32r pass, c = Xr @ W'r with W' = (w_in*mask)*boost folded on the
HOST, so the PE emits the BOOSTED scores directly and stage-2 skips the boost
multiply. Winner values are recovered as c * wz with wz = w_out_elem/boost (exact
algebra on the computed scores). Value path (zc, mask e, segment-sum input) runs
in bf16 for 2x DVE throughput; its ~2e-3 relative contribution is negligible vs
the ~1.1e-2 flip-driven error (measured, vs the 2e-2 tolerance).

Stage-2 runs ENTIRELY on the vector engine: with the mask-multiply on gpsimd the
strict-FIFO vector queue blocked ~2.3us/iter waiting on the cross-engine dep,
vector lag grew to the psum-pool depth, and the PE got throttled ~0.5us/iter plus
a ~30us drain tail. All-vector stage-2 (max, zc-mul, is_ge, mask-mul, segment
reduce ~6.2us/iter) pipelines cleanly under the 7.4us PE iteration.

Loop structure: chunk-pairs (j-chunks {0,1} then {2,3}), X batch-tiles loaded
once per half, prefetched 4 iterations ahead on the sync DMA queue (strips ride
the scalar queue; tables ship as one row + broadcast-DMA so they don't delay X).
The final iteration splits into two 1-bank psum chunks so chunk A's stage-2
drains under chunk B's matmuls. Measured ~503us (PE busy ~474us of a ~437us
f32r streaming floor; fp16/bf16 operand variants measured SLOWER on the PE
because their separate LDWEIGHTS does not hide as well as f32r's internal
4-byte weight load, and bf16 fails the error gate outright at 4.3e-2).
"""

import numpy as np

B, IN_DIM, OUT_DIM, DPC = 4096, 2048, 2048, 8
ND = OUT_DIM * DPC
NCORES = 8
UPC = OUT_DIM // NCORES          # units per core = 256
JPC = UPC * DPC                  # j' per core = 2048
CHUNK = 512                      # j' chunk width (64 units x 8 dendrites)
NCHUNK = JPC // CHUNK            # 4
BT = 128                         # batch tile
NBT = B // BT                    # 32
KT = 128                         # k tile
NKT = IN_DIM // KT               # 16
NSTRIP = NCHUNK * NKT            # 64
YW = CHUNK // DPC                # y columns per chunk = 64
BOOST_STRENGTH = 2.0

_prog_cache = {}
LAST_RESULTS = None


def _round_f32r(a):
    """Round fp32 -> f32r (11 explicit mantissa bits), RNE. Exact bit-twiddle."""
    u = a.view(np.uint32).astype(np.uint64)
    u = u + np.uint64(0xFFF) + ((u >> np.uint64(12)) & np.uint64(1))
    u = u & np.uint64(0xFFFFF000)
    return u.astype(np.uint32).view(np.float32)


_ldw_patched = False


def _enable_ldw_opt():
    """Flip walrus's --enable-ldw-opt to true (better LDWEIGHTS scheduling)."""
    global _ldw_patched
    if _ldw_patched:
        return
    from concourse import bass_utils as bu
    orig = bu.run_command

    def patched(argv, **kwargs):
        argv = ["--enable-ldw-opt=true" if a == "--enable-ldw-opt=false" else a
                for a in argv]
        return orig(argv, **kwargs)

    bu.run_command = patched
    _ldw_patched = True


def _build(has_bin, has_bout):
    import concourse.mybir as mybir
    import concourse.tile as tile
    from concourse import bacc

    _enable_ldw_opt()

    f32 = mybir.dt.float32
    f32r = mybir.dt.float32r
    bf16 = mybir.dt.bfloat16

    nc = bacc.Bacc("TRN2", target_bir_lowering=False, debug=False)
    XR_d = nc.dram_tensor("XR", [NBT, 128, NKT * BT], f32r, kind="ExternalInput").ap()
    WR_d = nc.dram_tensor("WR", [NSTRIP, 128, CHUNK], f32r, kind="ExternalInput").ap()
    WZ_d = nc.dram_tensor("WZ", [1, JPC], bf16, kind="ExternalInput").ap()
    if has_bin:
        binb_d = nc.dram_tensor("binb", [1, JPC], f32, kind="ExternalInput").ap()
    if has_bout:
        bout_d = nc.dram_tensor("bout", [1, NCHUNK * YW], f32, kind="ExternalInput").ap()
    Y_d = nc.dram_tensor("Y", [NCHUNK, B, YW], f32, kind="ExternalOutput").ap()

    with tile.TileContext(nc) as tc:
        with tc.tile_pool(name="tables", bufs=1) as tbl, \
             tc.tile_pool(name="wres", bufs=2) as wres, \
             tc.tile_pool(name="xsplit", bufs=5) as xsplit, \
             tc.tile_pool(name="ypool", bufs=3) as ypool, \
             tc.tile_pool(name="st2", bufs=2) as st2, \
             tc.tile_pool(name="psum", bufs=4, space="PSUM") as psum:

            xtiles = {}

            def emit_x(it):
                xr = xsplit.tile([128, NKT * BT], f32r, name=f"xr_{it}", tag="xr")
                nc.sync.dma_start(xr[:], XR_d[it % NBT])
                xtiles[it] = xr

            def xslice(xr, kt):
                return xr[:, kt*BT:(kt+1)*BT]

            strips = {}

            def emit_strip(w, kt, eng=None):
                wr = wres.tile([128, CHUNK], f32r, name=f"wr_{w}_{kt}",
                               tag=f"wr{w % 2}_{kt}")
                (eng or nc.scalar).dma_start(wr[:], WR_d[w * NKT + kt])
                strips[(w, kt)] = wr

            # X first on the sync queue so the first matmul isn't behind tables
            emit_x(0)
            emit_x(1)
            emit_x(2)
            # tables ship as one row + broadcast-DMA (4KB instead of 512KB)
            wz = tbl.tile([128, JPC], bf16, name="wz")
            nc.gpsimd.dma_start(out=wz[:], in_=WZ_d[:].to_broadcast((128, JPC)))
            emit_x(3)
            emit_x(4)
            if has_bin:
                binb = tbl.tile([128, JPC], f32, name="binb")
                nc.gpsimd.dma_start(out=binb[:], in_=binb_d[:].to_broadcast((128, JPC)))
            if has_bout:
                bot = tbl.tile([128, NCHUNK * YW], f32, name="bot")
                nc.gpsimd.dma_start(out=bot[:], in_=bout_d[:].to_broadcast((128, NCHUNK * YW)))
            # strips on the scalar queue, w-major to match consumption order
            for w in (0, 1):
                for kt in range(NKT):
                    emit_strip(w, kt)

            # warm the PE clock (HAM) during the dead window while X0/strips
            # are in flight: ~4.2us of dummy fp32 matmuls on SBUF scratch,
            # draining before X0 lands (~14us) so the real matmuls start warm
            scr_l = tbl.tile([128, BT], f32, name="scr_l")
            scr_r = tbl.tile([128, CHUNK], f32, name="scr_r")
            nc.gpsimd.memset(scr_l[:], 0.0)
            nc.gpsimd.memset(scr_r[:], 0.0)
            gw = psum.tile([128, 2 * CHUNK], f32, name="gwarm", tag="g")
            for _ in range(4):
                nc.tensor.matmul(gw[:, :CHUNK], scr_l[:], scr_r[:],
                                 start=True, stop=True)



            W2 = 2 * CHUNK

            def stage2_half(glap, w, i):
                # 512-wide stage 2 for one chunk (used to pipeline the final
                # iterations: chunk A's chain overlaps chunk B's matmuls)
                if has_bin:
                    gls = st2.tile([128, CHUNK], f32, name=f"gls_{w}_{i}", tag="gls")
                    nc.vector.tensor_add(gls[:], glap, binb[:, w*CHUNK:(w+1)*CHUNK])
                    glap = gls[:]
                mh = st2.tile([128, CHUNK // DPC], f32, name=f"mh_{w}_{i}", tag="mh")
                nc.vector.reduce_max(mh[:], glap.rearrange("p (u d) -> p u d", d=DPC),
                                     axis=mybir.AxisListType.X)
                zch = st2.tile([128, CHUNK], bf16, name=f"zch_{w}_{i}", tag="zch")
                nc.vector.tensor_mul(zch[:], glap, wz[:, w*CHUNK:(w+1)*CHUNK])
                eh = st2.tile([128, CHUNK], bf16, name=f"eh_{w}_{i}", tag="eh")
                mhb = mh[:].rearrange("p (u one) -> p u one", one=1).broadcast_to((128, CHUNK // DPC, DPC))
                nc.vector.tensor_tensor(eh[:].rearrange("p (u d) -> p u d", d=DPC),
                                        glap.rearrange("p (u d) -> p u d", d=DPC),
                                        mhb, op=mybir.AluOpType.is_ge)
                nc.vector.tensor_mul(zch[:], zch[:], eh[:])
                yh = ypool.tile([128, YW], f32, name=f"yh_{w}_{i}", tag="yh")
                ovh = zch[:].rearrange("p (s t q) -> p s q t", t=8, q=8)
                yvh = yh[:].rearrange("p (s q) -> p s q", q=8)
                nc.vector.reduce_sum(yvh, ovh, axis=mybir.AxisListType.X)
                if has_bout:
                    nc.vector.tensor_add(yh[:], yh[:], bot[:, w*YW:(w+1)*YW])
                nc.scalar.dma_start(Y_d[w, i*BT:(i+1)*BT, :], yh[:])

            for it in range(2 * NBT):
                half, i = divmod(it, NBT)
                w0 = 2 * half
                if it + 4 < 2 * NBT and (it + 4) not in xtiles:
                    emit_x(it + 4)
                xr = xtiles.pop(it)

                if it >= 2 * NBT - 2:
                    # final iterations: separate psum tiles per chunk so chunk
                    # A's stage 2 runs under chunk B's matmuls (short drain)
                    for wi in range(2):
                        glt = psum.tile([128, W2], f32, name=f"gl_{it}_{wi}", tag="g")
                        gl = glt[:, :CHUNK]
                        for kt in range(NKT):
                            nc.tensor.matmul(gl, xslice(xr, kt),
                                             strips[(w0 + wi, kt)][:],
                                             start=(kt == 0), stop=(kt == NKT - 1))
                        stage2_half(gl, w0 + wi, i)
                    continue

                # both chunks of the pair accumulate into one 2-bank psum tile
                g = psum.tile([128, W2], f32, name=f"g_{it}", tag="g")
                for wi in range(2):
                    gsub = g[:, wi*CHUNK:(wi+1)*CHUNK]
                    for kt in range(NKT):
                        nc.tensor.matmul(gsub, xslice(xr, kt),
                                         strips[(w0 + wi, kt)][:],
                                         start=(kt == 0), stop=(kt == NKT - 1))

                # spread next half's strip DMAs over this half
                if half == 0:
                    emit_strip(2 + i // NKT, i % NKT)

                # ---- stage 2: all on the vector engine so the chain never
                # blocks on a cross-engine dependency (psum readers first) ----
                # g already holds BOOSTED scores (boost folded into W on host)
                if has_bin:
                    gs = st2.tile([128, W2], f32, name=f"gs_{it}", tag="gs")
                    nc.vector.tensor_add(gs[:], g[:], binb[:, w0*CHUNK:(w0+2)*CHUNK])
                    gin = gs
                else:
                    gin = g
                m = st2.tile([128, W2 // DPC], f32, name=f"m_{it}", tag="m")
                nc.vector.reduce_max(m[:], gin[:].rearrange("p (u d) -> p u d", d=DPC),
                                     axis=mybir.AxisListType.X)
                zc = st2.tile([128, W2], bf16, name=f"zc_{it}", tag="zc")
                nc.vector.tensor_mul(zc[:], gin[:], wz[:, w0*CHUNK:(w0+2)*CHUNK])
                e = st2.tile([128, W2], bf16, name=f"e_{it}", tag="e")
                mb = m[:].rearrange("p (u one) -> p u one", one=1).broadcast_to((128, W2 // DPC, DPC))
                nc.vector.tensor_tensor(e[:].rearrange("p (u d) -> p u d", d=DPC),
                                        gin[:].rearrange("p (u d) -> p u d", d=DPC),
                                        mb, op=mybir.AluOpType.is_ge)
                nc.vector.tensor_mul(zc[:], zc[:], e[:])
                # y[p, wi*64 + 8s+q] = sum_t zc[wi*512 + 64s + 8t + q]
                y = ypool.tile([128, 2 * YW], f32, name=f"y_{it}", tag="y")
                ov = zc[:].rearrange("p (wi s t q) -> p wi s q t", wi=2, t=8, q=8)
                yv = y[:].rearrange("p (wi s q) -> p wi s q", wi=2, q=8)
                nc.vector.reduce_sum(yv, ov, axis=mybir.AxisListType.X)
                if has_bout:
                    nc.vector.tensor_add(y[:], y[:], bot[:, w0*YW:(w0+2)*YW])
                nc.scalar.dma_start(Y_d[w0, i*BT:(i+1)*BT, :], y[:, :YW])
                nc.scalar.dma_start(Y_d[w0 + 1, i*BT:(i+1)*BT, :], y[:, YW:])

    nc.compile()
    return nc


def _tile_x(a):
    """[B, IN_DIM] -> [NBT, 128(p=k%128), NKT*BT] preserving dtype."""
    return np.ascontiguousarray(
        a.reshape(NBT, BT, NKT, 128).transpose(0, 3, 2, 1).reshape(NBT, 128, -1))


def _tile_w(a):
    """[IN_DIM, JPC] -> [NCHUNK*NKT, 128, CHUNK]."""
    return np.ascontiguousarray(
        a.reshape(NKT, 128, NCHUNK, CHUNK).transpose(2, 0, 1, 3).reshape(NSTRIP, 128, CHUNK))


def kernel(x, w_in, b_in, w_in_mask, w_out, b_out, duty_cycle):
    from concourse.bass_utils import run_bass_kernel_spmd
    global LAST_RESULTS

    x = np.ascontiguousarray(x, dtype=np.float32)
    w_in = np.asarray(w_in, dtype=np.float32)
    w_in_mask = np.asarray(w_in_mask, dtype=np.float32)
    w_out = np.asarray(w_out, dtype=np.float32)
    b_in = np.asarray(b_in, dtype=np.float32)
    b_out = np.asarray(b_out, dtype=np.float32)
    duty_cycle = np.asarray(duty_cycle, dtype=np.float32)
    assert x.shape == (B, IN_DIM) and w_in.shape == (ND, IN_DIM)

    has_bin = bool(np.any(b_in))
    has_bout = bool(np.any(b_out))

    key = (has_bin, has_bout)
    if key not in _prog_cache:
        _prog_cache[key] = _build(has_bin, has_bout)
    nc = _prog_cache[key]

    # ---- host-side operand prep: boost folding + f32r rounding + tiling ----
    XRt = _tile_x(_round_f32r(x))                         # [NBT, 128, NKT*BT]

    # w_in[d*OUT + c*UPC + u', k] -> per-core [k, j'=u'*8+d] via reshape/transpose
    Wmask = w_in * w_in_mask                              # [ND, IN_DIM]
    w4 = Wmask.reshape(DPC, NCORES, UPC, IN_DIM)          # [d, c, u', k]
    wof = w_out.reshape(-1)
    boost_all = np.exp((1.0 / DPC - duty_cycle) * BOOST_STRENGTH)  # [DPC, OUT_DIM]

    uprime = np.arange(UPC)
    dd = np.arange(DPC)
    jp_u = np.repeat(uprime, DPC)                         # u'(j') ; j' = u'*8 + d
    jp_d = np.tile(dd, UPC)                               # d(j')

    in_maps = []
    for c in range(NCORES):
        rows = jp_d * OUT_DIM + c * UPC + jp_u            # global w_in row per j'
        bj = boost_all[jp_d, c * UPC + jp_u].astype(np.float32)        # [JPC]
        Wm = np.ascontiguousarray(w4[:, c].transpose(2, 1, 0).reshape(IN_DIM, JPC))
        WRt = _tile_w(_round_f32r(Wm * bj[None, :]))      # boosted weights
        v = jp_d * (OUT_DIM // DPC) + c * (UPC // DPC) + (jp_u // DPC)  # d*256 + c*32 + u'//8
        t = jp_u % DPC
        import ml_dtypes
        wz = (wof[v * ND + v * DPC + t] / bj).astype(ml_dtypes.bfloat16)
        im = {"XR": XRt, "WR": WRt, "WZ": np.ascontiguousarray(wz.reshape(1, JPC))}
        if has_bin:
            im["binb"] = np.ascontiguousarray(
                (b_in[rows] * bj).astype(np.float32).reshape(1, JPC))
        if has_bout:
            # bout4[w*64 + s*8 + q] = b_out[v], v = q*256 + c*32 + 8w + s
            wq = np.arange(NCHUNK * YW)
            wi, si, qi = wq // YW, (wq % YW) // 8, wq % 8
            vv = qi * (OUT_DIM // DPC) + c * (UPC // DPC) + 8 * wi + si
            im["bout"] = np.ascontiguousarray(
                b_out[vv].astype(np.float32).reshape(1, NCHUNK * YW))
        in_maps.append(im)

    import os
    trace = bool(os.environ.get("KERNEL_TRACE"))
    last_err = None
    for _attempt in range(3):
        try:
            res = run_bass_kernel_spmd(nc, in_maps, list(range(NCORES)), trace=trace)
            break
        except Exception as err:  # rare transient device fault on first execute
            last_err = err
            import time as _time
            _time.sleep(2.0)
    else:
        raise last_err
    LAST_RESULTS = res

    # Y4[w, b, s*8+q] (per core) -> y[b, q*256 + c*32 + 8w + s]
    Yc = np.stack([res.results[c]["Y"] for c in range(NCORES)], axis=0)  # [8, NCHUNK, B, 64]
    Yc = Yc.reshape(NCORES, NCHUNK, B, 8, 8)             # [c, w, b, s, q]
    y = Yc.transpose(2, 4, 0, 1, 3).reshape(B, OUT_DIM)  # [b, q, c, w, s] -> v = q*256+c*32+8w+s
    return np.ascontiguousarray(y)


# revision 40
# speedup vs baseline: 1.0060x; 1.0060x over previous
"""DendriteLayer Trainium2 kernel.

Math (reference): out0 = x @ (w_in*w_in_mask).T + b_in; a = out0.reshape(B, dpc, out_dim);
winner = argmax_d(a * boost); out1 = a * one_hot(winner); y = out1f @ (w_out*dend_mask).T + b_out.

Sharding: 8 cores, core c owns global units u in [c*256, (c+1)*256) (all dpc=8 dendrites)
and output columns v with (v % 256) in [c*32, (c+1)*32). Both k-winners and the
block-diagonal output stage are then fully local to a core (no collectives).

Per-core j' layout is u'-major interleaved: j' = u'*8 + d, so the 8 dendrites of a
unit are consecutive, and each 512-wide chunk of j' is self-contained for both the
k-winners (max over d) and the output segment-sums.

Matmul: single f32r pass, c = Xr @ W'r with W' = (w_in*mask)*boost folded on the
HOST, so the PE emits the BOOSTED scores directly and stage-2 skips the boost
multiply. Winner values are recovered as c * wz with wz = w_out_elem/boost (exact
algebra on the computed scores). Value path (zc, mask e, segment-sum input) runs
in bf16 for 2x DVE throughput; its ~2e-3 relative contribution is negligible vs
the ~1.1e-2 flip-driven error (measured, vs the 2e-2 tolerance).

Stage-2 runs ENTIRELY on the vector engine: with the mask-multiply on gpsimd the
strict-FIFO vector queue blocked ~2.3us/iter waiting on the cross-engine dep,
vector lag grew to the psum-pool depth, and the PE got throttled ~0.5us/iter plus
a ~30us drain tail. All-vector stage-2 (max, zc-mul, is_ge, mask-mul, segment
reduce ~6.2us/iter) pipelines cleanly under the 7.4us PE iteration.

Loop structure: chunk-pairs (j-chunks {0,1} then {2,3}), X batch-tiles loaded
once per half, prefetched 4 iterations ahead on the sync DMA queue (strips ride
the scalar queue; tables ship as one row + broadcast-DMA so they don't delay X).
The final iteration splits into two 1-bank psum chunks so chunk A's stage-2
drains under chunk B's matmuls. Measured ~503us (PE busy ~474us of a ~437us
f32r streaming floor; fp16/bf16 operand variants measured SLOWER on the PE
because their separate LDWEIGHTS does not hide as well as f32r's internal
4-byte weight load, and bf16 fails the error gate outright at 4.3e-2).
"""

import numpy as np

B, IN_DIM, OUT_DIM, DPC = 4096, 2048, 2048, 8
ND = OUT_DIM * DPC
NCORES = 8
UPC = OUT_DIM // NCORES          # units per core = 256
JPC = UPC * DPC                  # j' per core = 2048
CHUNK = 512                      # j' chunk width (64 units x 8 dendrites)
NCHUNK = JPC // CHUNK            # 4
BT = 128                         # batch tile
NBT = B // BT                    # 32
KT = 128                         # k tile
NKT = IN_DIM // KT               # 16
NSTRIP = NCHUNK * NKT            # 64
YW = CHUNK // DPC                # y columns per chunk = 64
BOOST_STRENGTH = 2.0

_prog_cache = {}
LAST_RESULTS = None


def _round_f32r(a):
    """Round fp32 -> f32r (11 explicit mantissa bits), RNE. Exact bit-twiddle."""
    u = a.view(np.uint32).astype(np.uint64)
    u = u + np.uint64(0xFFF) + ((u >> np.uint64(12)) & np.uint64(1))
    u = u & np.uint64(0xFFFFF000)
    return u.astype(np.uint32).view(np.float32)


_ldw_patched = False


def _enable_ldw_opt():
    """Flip walrus's --enable-ldw-opt to true (better LDWEIGHTS scheduling)."""
    global _ldw_patched
    if _ldw_patched:
        return
    from concourse import bass_utils as bu
    orig = bu.run_command

    def patched(argv, **kwargs):
        argv = ["--enable-ldw-opt=true" if a == "--enable-ldw-opt=false" else a
                for a in argv]
        return orig(argv, **kwargs)

    bu.run_command = patched
    _ldw_patched = True


def _build(has_bin, has_bout):
    import concourse.mybir as mybir
    import concourse.tile as tile
    from concourse import bacc

    _enable_ldw_opt()

    f32 = mybir.dt.float32
    f32r = mybir.dt.float32r
    bf16 = mybir.dt.bfloat16

    nc = bacc.Bacc("TRN2", target_bir_lowering=False, debug=False)
    XR_d = nc.dram_tensor("XR", [NBT, 128, NKT * BT], f32r, kind="ExternalInput").ap()
    WR_d = nc.dram_tensor("WR", [NSTRIP, 128, CHUNK], f32r, kind="ExternalInput").ap()
    WZ_d = nc.dram_tensor("WZ", [1, JPC], bf16, kind="ExternalInput").ap()
    if has_bin:
        binb_d = nc.dram_tensor("binb", [1, JPC], f32, kind="ExternalInput").ap()
    if has_bout:
        bout_d = nc.dram_tensor("bout", [1, NCHUNK * YW], f32, kind="ExternalInput").ap()
    Y_d = nc.dram_tensor("Y", [NCHUNK, B, YW], f32, kind="ExternalOutput").ap()

    with tile.TileContext(nc) as tc:
        with tc.tile_pool(name="tables", bufs=1) as tbl, \
             tc.tile_pool(name="wres", bufs=2) as wres, \
             tc.tile_pool(name="xsplit", bufs=5) as xsplit, \
             tc.tile_pool(name="ypool", bufs=3) as ypool, \
             tc.tile_pool(name="st2", bufs=2) as st2, \
             tc.tile_pool(name="psum", bufs=4, space="PSUM") as psum:

            xtiles = {}

            def emit_x(it):
                xr = xsplit.tile([128, NKT * BT], f32r, name=f"xr_{it}", tag="xr")
                nc.sync.dma_start(xr[:], XR_d[it % NBT])
                xtiles[it] = xr

            def xslice(xr, kt):
                return xr[:, kt*BT:(kt+1)*BT]

            strips = {}

            def emit_strip(w, kt, eng=None):
                wr = wres.tile([128, CHUNK], f32r, name=f"wr_{w}_{kt}",
                               tag=f"wr{w % 2}_{kt}")
                (eng or nc.scalar).dma_start(wr[:], WR_d[w * NKT + kt])
                strips[(w, kt)] = wr

            # X first on the sync queue so the first matmul isn't behind tables
            emit_x(0)
            emit_x(1)
            emit_x(2)
            # tables ship as one row + broadcast-DMA (4KB instead of 512KB)
            wz = tbl.tile([128, JPC], bf16, name="wz")
            nc.gpsimd.dma_start(out=wz[:], in_=WZ_d[:].to_broadcast((128, JPC)))
            emit_x(3)
            emit_x(4)
            if has_bin:
                binb = tbl.tile([128, JPC], f32, name="binb")
                nc.gpsimd.dma_start(out=binb[:], in_=binb_d[:].to_broadcast((128, JPC)))
            if has_bout:
                bot = tbl.tile([128, NCHUNK * YW], f32, name="bot")
                nc.gpsimd.dma_start(out=bot[:], in_=bout_d[:].to_broadcast((128, NCHUNK * YW)))
            # strips on the scalar queue, w-major to match consumption order
            for w in (0, 1):
                for kt in range(NKT):
                    emit_strip(w, kt)




            W2 = 2 * CHUNK

            def stage2_half(glap, w, i):
                # 512-wide stage 2 for one chunk (used to pipeline the final
                # iterations: chunk A's chain overlaps chunk B's matmuls)
                if has_bin:
                    gls = st2.tile([128, CHUNK], f32, name=f"gls_{w}_{i}", tag="gls")
                    nc.vector.tensor_add(gls[:], glap, binb[:, w*CHUNK:(w+1)*CHUNK])
                    glap = gls[:]
                mh = st2.tile([128, CHUNK // DPC], f32, name=f"mh_{w}_{i}", tag="mh")
                nc.vector.reduce_max(mh[:], glap.rearrange("p (u d) -> p u d", d=DPC),
                                     axis=mybir.AxisListType.X)
                zch = st2.tile([128, CHUNK], bf16, name=f"zch_{w}_{i}", tag="zch")
                nc.vector.tensor_mul(zch[:], glap, wz[:, w*CHUNK:(w+1)*CHUNK])
                eh = st2.tile([128, CHUNK], bf16, name=f"eh_{w}_{i}", tag="eh")
                mhb = mh[:].rearrange("p (u one) -> p u one", one=1).broadcast_to((128, CHUNK // DPC, DPC))
                nc.vector.tensor_tensor(eh[:].rearrange("p (u d) -> p u d", d=DPC),
                                        glap.rearrange("p (u d) -> p u d", d=DPC),
                                        mhb, op=mybir.AluOpType.is_ge)
                nc.vector.tensor_mul(zch[:], zch[:], eh[:])
                yh = ypool.tile([128, YW], f32, name=f"yh_{w}_{i}", tag="yh")
                ovh = zch[:].rearrange("p (s t q) -> p s q t", t=8, q=8)
                yvh = yh[:].rearrange("p (s q) -> p s q", q=8)
                nc.vector.reduce_sum(yvh, ovh, axis=mybir.AxisListType.X)
                if has_bout:
                    nc.vector.tensor_add(yh[:], yh[:], bot[:, w*YW:(w+1)*YW])
                nc.scalar.dma_start(Y_d[w, i*BT:(i+1)*BT, :], yh[:])

            for it in range(2 * NBT):
                half, i = divmod(it, NBT)
                w0 = 2 * half
                if it + 4 < 2 * NBT and (it + 4) not in xtiles:
                    emit_x(it + 4)
                xr = xtiles.pop(it)

                if it >= 2 * NBT - 2:
                    # final iterations: separate psum tiles per chunk so chunk
                    # A's stage 2 runs under chunk B's matmuls (short drain)
                    for wi in range(2):
                        glt = psum.tile([128, W2], f32, name=f"gl_{it}_{wi}", tag="g")
                        gl = glt[:, :CHUNK]
                        for kt in range(NKT):
                            nc.tensor.matmul(gl, xslice(xr, kt),
                                             strips[(w0 + wi, kt)][:],
                                             start=(kt == 0), stop=(kt == NKT - 1))
                        stage2_half(gl, w0 + wi, i)
                    continue

                # both chunks of the pair accumulate into one 2-bank psum tile
                g = psum.tile([128, W2], f32, name=f"g_{it}", tag="g")
                for wi in range(2):
                    gsub = g[:, wi*CHUNK:(wi+1)*CHUNK]
                    for kt in range(NKT):
                        nc.tensor.matmul(gsub, xslice(xr, kt),
                                         strips[(w0 + wi, kt)][:],
                                         start=(kt == 0), stop=(kt == NKT - 1))

                # spread next half's strip DMAs over this half
                if half == 0:
                    emit_strip(2 + i // NKT, i % NKT)

                # ---- stage 2: all on the vector engine so the chain never
                # blocks on a cross-engine dependency (psum readers first) ----
                # g already holds BOOSTED scores (boost folded into W on host)
                if has_bin:
                    gs = st2.tile([128, W2], f32, name=f"gs_{it}", tag="gs")
                    nc.vector.tensor_add(gs[:], g[:], binb[:, w0*CHUNK:(w0+2)*CHUNK])
                    gin = gs
                else:
                    gin = g
                m = st2.tile([128, W2 // DPC], f32, name=f"m_{it}", tag="m")
                nc.vector.reduce_max(m[:], gin[:].rearrange("p (u d) -> p u d", d=DPC),
                                     axis=mybir.AxisListType.X)
                zc = st2.tile([128, W2], bf16, name=f"zc_{it}", tag="zc")
                nc.vector.tensor_mul(zc[:], gin[:], wz[:, w0*CHUNK:(w0+2)*CHUNK])
                e = st2.tile([128, W2], bf16, name=f"e_{it}", tag="e")
                mb = m[:].rearrange("p (u one) -> p u one", one=1).broadcast_to((128, W2 // DPC, DPC))
                nc.vector.tensor_tensor(e[:].rearrange("p (u d) -> p u d", d=DPC),
                                        gin[:].rearrange("p (u d) -> p u d", d=DPC),
                                        mb, op=mybir.AluOpType.is_ge)
                nc.vector.tensor_mul(zc[:], zc[:], e[:])
                # y[p, wi*64 + 8s+q] = sum_t zc[wi*512 + 64s + 8t + q]
                y = ypool.tile([128, 2 * YW], f32, name=f"y_{it}", tag="y")
                ov = zc[:].rearrange("p (wi s t q) -> p wi s q t", wi=2, t=8, q=8)
                yv = y[:].rearrange("p (wi s q) -> p wi s q", wi=2, q=8)
                nc.vector.reduce_sum(yv, ov, axis=mybir.AxisListType.X)
                if has_bout:
                    nc.vector.tensor_add(y[:], y[:], bot[:, w0*YW:(w0+2)*YW])
                nc.scalar.dma_start(Y_d[w0, i*BT:(i+1)*BT, :], y[:, :YW])
                nc.scalar.dma_start(Y_d[w0 + 1, i*BT:(i+1)*BT, :], y[:, YW:])

    nc.compile()
    return nc


def _tile_x(a):
    """[B, IN_DIM] -> [NBT, 128(p=k%128), NKT*BT] preserving dtype."""
    return np.ascontiguousarray(
        a.reshape(NBT, BT, NKT, 128).transpose(0, 3, 2, 1).reshape(NBT, 128, -1))


def _tile_w(a):
    """[IN_DIM, JPC] -> [NCHUNK*NKT, 128, CHUNK]."""
    return np.ascontiguousarray(
        a.reshape(NKT, 128, NCHUNK, CHUNK).transpose(2, 0, 1, 3).reshape(NSTRIP, 128, CHUNK))


def kernel(x, w_in, b_in, w_in_mask, w_out, b_out, duty_cycle):
    from concourse.bass_utils import run_bass_kernel_spmd
    global LAST_RESULTS

    x = np.ascontiguousarray(x, dtype=np.float32)
    w_in = np.asarray(w_in, dtype=np.float32)
    w_in_mask = np.asarray(w_in_mask, dtype=np.float32)
    w_out = np.asarray(w_out, dtype=np.float32)
    b_in = np.asarray(b_in, dtype=np.float32)
    b_out = np.asarray(b_out, dtype=np.float32)
    duty_cycle = np.asarray(duty_cycle, dtype=np.float32)
    assert x.shape == (B, IN_DIM) and w_in.shape == (ND, IN_DIM)

    has_bin = bool(np.any(b_in))
    has_bout = bool(np.any(b_out))

    key = (has_bin, has_bout)
    if key not in _prog_cache:
        _prog_cache[key] = _build(has_bin, has_bout)
    nc = _prog_cache[key]

    # ---- host-side operand prep: boost folding + f32r rounding + tiling ----
    XRt = _tile_x(_round_f32r(x))                         # [NBT, 128, NKT*BT]

    # w_in[d*OUT + c*UPC + u', k] -> per-core [k, j'=u'*8+d] via reshape/transpose
    Wmask = w_in * w_in_mask                              # [ND, IN_DIM]
    w4 = Wmask.reshape(DPC, NCORES, UPC, IN_DIM)          # [d, c, u', k]
    wof = w_out.reshape(-1)
    boost_all = np.exp((1.0 / DPC - duty_cycle) * BOOST_STRENGTH)  # [DPC, OUT_DIM]

    uprime = np.arange(UPC)
    dd = np.arange(DPC)
    jp_u = np.repeat(uprime, DPC)                         # u'(j') ; j' = u'*8 + d
    jp_d = np.tile(dd, UPC)                               # d(j')

    in_maps = []
    for c in range(NCORES):
        rows = jp_d * OUT_DIM + c * UPC + jp_u            # global w_in row per j'
        bj = boost_all[jp_d, c * UPC + jp_u].astype(np.float32)        # [JPC]
        Wm = np.ascontiguousarray(w4[:, c].transpose(2, 1, 0).reshape(IN_DIM, JPC))
        WRt = _tile_w(_round_f32r(Wm * bj[None, :]))      # boosted weights
        v = jp_d * (OUT_DIM // DPC) + c * (UPC // DPC) + (jp_u // DPC)  # d*256 + c*32 + u'//8
        t = jp_u % DPC
        import ml_dtypes
        wz = (wof[v * ND + v * DPC + t] / bj).astype(ml_dtypes.bfloat16)
        im = {"XR": XRt, "WR": WRt, "WZ": np.ascontiguousarray(wz.reshape(1, JPC))}
        if has_bin:
            im["binb"] = np.ascontiguousarray(
                (b_in[rows] * bj).astype(np.float32).reshape(1, JPC))
        if has_bout:
            # bout4[w*64 + s*8 + q] = b_out[v], v = q*256 + c*32 + 8w + s
            wq = np.arange(NCHUNK * YW)
            wi, si, qi = wq // YW, (wq % YW) // 8, wq % 8
            vv = qi * (OUT_DIM // DPC) + c * (UPC // DPC) + 8 * wi + si
            im["bout"] = np.ascontiguousarray(
                b_out[vv].astype(np.float32).reshape(1, NCHUNK * YW))
        in_maps.append(im)

    import os
    trace = bool(os.environ.get("KERNEL_TRACE"))
    last_err = None
    for _attempt in range(3):
        try:
            res = run_bass_kernel_spmd(nc, in_maps, list(range(NCORES)), trace=trace)
            break
        except Exception as err:  # rare transient device fault on first execute
            last_err = err
            import time as _time
            _time.sleep(2.0)
    else:
        raise last_err
    LAST_RESULTS = res

    # Y4[w, b, s*8+q] (per core) -> y[b, q*256 + c*32 + 8w + s]
    Yc = np.stack([res.results[c]["Y"] for c in range(NCORES)], axis=0)  # [8, NCHUNK, B, 64]
    Yc = Yc.reshape(NCORES, NCHUNK, B, 8, 8)             # [c, w, b, s, q]
    y = Yc.transpose(2, 4, 0, 1, 3).reshape(B, OUT_DIM)  # [b, q, c, w, s] -> v = q*256+c*32+8w+s
    return np.ascontiguousarray(y)


# revision 43
# speedup vs baseline: 1.0064x; 1.0004x over previous
"""DendriteLayer Trainium2 kernel.

Math (reference): out0 = x @ (w_in*w_in_mask).T + b_in; a = out0.reshape(B, dpc, out_dim);
winner = argmax_d(a * boost); out1 = a * one_hot(winner); y = out1f @ (w_out*dend_mask).T + b_out.

Sharding: 8 cores, core c owns global units u in [c*256, (c+1)*256) (all dpc=8 dendrites)
and output columns v with (v % 256) in [c*32, (c+1)*32). Both k-winners and the
block-diagonal output stage are then fully local to a core (no collectives).

Per-core j' layout is u'-major interleaved: j' = u'*8 + d, so the 8 dendrites of a
unit are consecutive, and each 512-wide chunk of j' is self-contained for both the
k-winners (max over d) and the output segment-sums.

Matmul: single f32r pass, c = Xr @ W'r with W' = (w_in*mask)*boost folded on the
HOST, so the PE emits the BOOSTED scores directly and stage-2 skips the boost
multiply. Winner values are recovered as c * wz with wz = w_out_elem/boost (exact
algebra on the computed scores). Value path (zc, mask e, segment-sum input) runs
in bf16 for 2x DVE throughput; its ~2e-3 relative contribution is negligible vs
the ~1.1e-2 flip-driven error (measured, vs the 2e-2 tolerance).

Stage-2 runs ENTIRELY on the vector engine: with the mask-multiply on gpsimd the
strict-FIFO vector queue blocked ~2.3us/iter waiting on the cross-engine dep,
vector lag grew to the psum-pool depth, and the PE got throttled ~0.5us/iter plus
a ~30us drain tail. All-vector stage-2 (max, zc-mul, is_ge, mask-mul, segment
reduce ~6.2us/iter) pipelines cleanly under the 7.4us PE iteration.

Loop structure: chunk-pairs (j-chunks {0,1} then {2,3}), X batch-tiles loaded
once per half, prefetched 4 iterations ahead on the sync DMA queue (strips ride
the scalar queue; tables ship as one row + broadcast-DMA so they don't delay X).
The final iteration splits into two 1-bank psum chunks so chunk A's stage-2
drains under chunk B's matmuls. Measured ~503us (PE busy ~474us of a ~437us
f32r streaming floor; fp16/bf16 operand variants measured SLOWER on the PE
because their separate LDWEIGHTS does not hide as well as f32r's internal
4-byte weight load, and bf16 fails the error gate outright at 4.3e-2).
"""

import numpy as np

B, IN_DIM, OUT_DIM, DPC = 4096, 2048, 2048, 8
ND = OUT_DIM * DPC
NCORES = 8
UPC = OUT_DIM // NCORES          # units per core = 256
JPC = UPC * DPC                  # j' per core = 2048
CHUNK = 512                      # j' chunk width (64 units x 8 dendrites)
NCHUNK = JPC // CHUNK            # 4
BT = 128                         # batch tile
NBT = B // BT                    # 32
KT = 128                         # k tile
NKT = IN_DIM // KT               # 16
NSTRIP = NCHUNK * NKT            # 64
YW = CHUNK // DPC                # y columns per chunk = 64
BOOST_STRENGTH = 2.0

_prog_cache = {}
LAST_RESULTS = None


def _round_f32r(a):
    """Round fp32 -> f32r (11 explicit mantissa bits), RNE. Exact bit-twiddle."""
    u = a.view(np.uint32).astype(np.uint64)
    u = u + np.uint64(0xFFF) + ((u >> np.uint64(12)) & np.uint64(1))
    u = u & np.uint64(0xFFFFF000)
    return u.astype(np.uint32).view(np.float32)


_ldw_patched = False


def _enable_ldw_opt():
    """Flip walrus's --enable-ldw-opt to true (better LDWEIGHTS scheduling)."""
    global _ldw_patched
    if _ldw_patched:
        return
    from concourse import bass_utils as bu
    orig = bu.run_command

    def patched(argv, **kwargs):
        argv = ["--enable-ldw-opt=true" if a == "--enable-ldw-opt=false" else a
                for a in argv]
        return orig(argv, **kwargs)

    bu.run_command = patched
    _ldw_patched = True


def _build(has_bin, has_bout):
    import concourse.mybir as mybir
    import concourse.tile as tile
    from concourse import bacc

    _enable_ldw_opt()

    f32 = mybir.dt.float32
    f32r = mybir.dt.float32r
    bf16 = mybir.dt.bfloat16

    nc = bacc.Bacc("TRN2", target_bir_lowering=False, debug=False)
    XR_d = nc.dram_tensor("XR", [NBT, 128, NKT * BT], f32r, kind="ExternalInput").ap()
    WR_d = nc.dram_tensor("WR", [NSTRIP, 128, CHUNK], f32r, kind="ExternalInput").ap()
    WZ_d = nc.dram_tensor("WZ", [1, JPC], bf16, kind="ExternalInput").ap()
    if has_bin:
        binb_d = nc.dram_tensor("binb", [1, JPC], f32, kind="ExternalInput").ap()
    if has_bout:
        bout_d = nc.dram_tensor("bout", [1, NCHUNK * YW], f32, kind="ExternalInput").ap()
    Y_d = nc.dram_tensor("Y", [NCHUNK, B, YW], f32, kind="ExternalOutput").ap()

    with tile.TileContext(nc) as tc:
        with tc.tile_pool(name="tables", bufs=1) as tbl, \
             tc.tile_pool(name="wres", bufs=2) as wres, \
             tc.tile_pool(name="xsplit", bufs=5) as xsplit, \
             tc.tile_pool(name="ypool", bufs=3) as ypool, \
             tc.tile_pool(name="st2", bufs=2) as st2, \
             tc.tile_pool(name="psum", bufs=4, space="PSUM") as psum:

            xtiles = {}

            def emit_x(it, eng=None):
                xr = xsplit.tile([128, NKT * BT], f32r, name=f"xr_{it}", tag="xr")
                (eng or nc.sync).dma_start(xr[:], XR_d[it % NBT])
                xtiles[it] = xr

            def xslice(xr, kt):
                return xr[:, kt*BT:(kt+1)*BT]

            strips = {}

            def emit_strip(w, kt, eng=None):
                wr = wres.tile([128, CHUNK], f32r, name=f"wr_{w}_{kt}",
                               tag=f"wr{w % 2}_{kt}")
                (eng or nc.scalar).dma_start(wr[:], WR_d[w * NKT + kt])
                strips[(w, kt)] = wr

            # X0 leads the scalar queue (it starts earlier than sync);
            # remaining X tiles ride the sync queue ahead of the tables
            emit_x(0, nc.scalar)
            emit_x(1)
            emit_x(2)
            # tables ship as one row + broadcast-DMA (4KB instead of 512KB)
            wz = tbl.tile([128, JPC], bf16, name="wz")
            nc.gpsimd.dma_start(out=wz[:], in_=WZ_d[:].to_broadcast((128, JPC)))
            emit_x(3)
            emit_x(4)
            if has_bin:
                binb = tbl.tile([128, JPC], f32, name="binb")
                nc.gpsimd.dma_start(out=binb[:], in_=binb_d[:].to_broadcast((128, JPC)))
            if has_bout:
                bot = tbl.tile([128, NCHUNK * YW], f32, name="bot")
                nc.gpsimd.dma_start(out=bot[:], in_=bout_d[:].to_broadcast((128, NCHUNK * YW)))
            # strips on the scalar queue, w-major to match consumption order
            for w in (0, 1):
                for kt in range(NKT):
                    emit_strip(w, kt)




            W2 = 2 * CHUNK

            def stage2_half(glap, w, i):
                # 512-wide stage 2 for one chunk (used to pipeline the final
                # iterations: chunk A's chain overlaps chunk B's matmuls)
                if has_bin:
                    gls = st2.tile([128, CHUNK], f32, name=f"gls_{w}_{i}", tag="gls")
                    nc.vector.tensor_add(gls[:], glap, binb[:, w*CHUNK:(w+1)*CHUNK])
                    glap = gls[:]
                mh = st2.tile([128, CHUNK // DPC], f32, name=f"mh_{w}_{i}", tag="mh")
                nc.vector.reduce_max(mh[:], glap.rearrange("p (u d) -> p u d", d=DPC),
                                     axis=mybir.AxisListType.X)
                zch = st2.tile([128, CHUNK], bf16, name=f"zch_{w}_{i}", tag="zch")
                nc.vector.tensor_mul(zch[:], glap, wz[:, w*CHUNK:(w+1)*CHUNK])
                eh = st2.tile([128, CHUNK], bf16, name=f"eh_{w}_{i}", tag="eh")
                mhb = mh[:].rearrange("p (u one) -> p u one", one=1).broadcast_to((128, CHUNK // DPC, DPC))
                nc.vector.tensor_tensor(eh[:].rearrange("p (u d) -> p u d", d=DPC),
                                        glap.rearrange("p (u d) -> p u d", d=DPC),
                                        mhb, op=mybir.AluOpType.is_ge)
                nc.vector.tensor_mul(zch[:], zch[:], eh[:])
                yh = ypool.tile([128, YW], f32, name=f"yh_{w}_{i}", tag="yh")
                ovh = zch[:].rearrange("p (s t q) -> p s q t", t=8, q=8)
                yvh = yh[:].rearrange("p (s q) -> p s q", q=8)
                nc.vector.reduce_sum(yvh, ovh, axis=mybir.AxisListType.X)
                if has_bout:
                    nc.vector.tensor_add(yh[:], yh[:], bot[:, w*YW:(w+1)*YW])
                nc.scalar.dma_start(Y_d[w, i*BT:(i+1)*BT, :], yh[:])

            for it in range(2 * NBT):
                half, i = divmod(it, NBT)
                w0 = 2 * half
                if it + 4 < 2 * NBT and (it + 4) not in xtiles:
                    emit_x(it + 4)
                xr = xtiles.pop(it)

                if it >= 2 * NBT - 3:
                    # final iterations: separate psum tiles per chunk so chunk
                    # A's stage 2 runs under chunk B's matmuls (short drain)
                    for wi in range(2):
                        glt = psum.tile([128, W2], f32, name=f"gl_{it}_{wi}", tag="g")
                        gl = glt[:, :CHUNK]
                        for kt in range(NKT):
                            nc.tensor.matmul(gl, xslice(xr, kt),
                                             strips[(w0 + wi, kt)][:],
                                             start=(kt == 0), stop=(kt == NKT - 1))
                        stage2_half(gl, w0 + wi, i)
                    continue

                # both chunks of the pair accumulate into one 2-bank psum tile
                g = psum.tile([128, W2], f32, name=f"g_{it}", tag="g")
                for wi in range(2):
                    gsub = g[:, wi*CHUNK:(wi+1)*CHUNK]
                    for kt in range(NKT):
                        nc.tensor.matmul(gsub, xslice(xr, kt),
                                         strips[(w0 + wi, kt)][:],
                                         start=(kt == 0), stop=(kt == NKT - 1))

                # spread next half's strip DMAs over this half
                if half == 0:
                    emit_strip(2 + i // NKT, i % NKT)

                # ---- stage 2: all on the vector engine so the chain never
                # blocks on a cross-engine dependency (psum readers first) ----
                # g already holds BOOSTED scores (boost folded into W on host)
                if has_bin:
                    gs = st2.tile([128, W2], f32, name=f"gs_{it}", tag="gs")
                    nc.vector.tensor_add(gs[:], g[:], binb[:, w0*CHUNK:(w0+2)*CHUNK])
                    gin = gs
                else:
                    gin = g
                m = st2.tile([128, W2 // DPC], f32, name=f"m_{it}", tag="m")
                nc.vector.reduce_max(m[:], gin[:].rearrange("p (u d) -> p u d", d=DPC),
                                     axis=mybir.AxisListType.X)
                zc = st2.tile([128, W2], bf16, name=f"zc_{it}", tag="zc")
                nc.vector.tensor_mul(zc[:], gin[:], wz[:, w0*CHUNK:(w0+2)*CHUNK])
                e = st2.tile([128, W2], bf16, name=f"e_{it}", tag="e")
                mb = m[:].rearrange("p (u one) -> p u one", one=1).broadcast_to((128, W2 // DPC, DPC))
                nc.vector.tensor_tensor(e[:].rearrange("p (u d) -> p u d", d=DPC),
                                        gin[:].rearrange("p (u d) -> p u d", d=DPC),
                                        mb, op=mybir.AluOpType.is_ge)
                nc.vector.tensor_mul(zc[:], zc[:], e[:])
                # y[p, wi*64 + 8s+q] = sum_t zc[wi*512 + 64s + 8t + q]
                y = ypool.tile([128, 2 * YW], f32, name=f"y_{it}", tag="y")
                ov = zc[:].rearrange("p (wi s t q) -> p wi s q t", wi=2, t=8, q=8)
                yv = y[:].rearrange("p (wi s q) -> p wi s q", wi=2, q=8)
                nc.vector.reduce_sum(yv, ov, axis=mybir.AxisListType.X)
                if has_bout:
                    nc.vector.tensor_add(y[:], y[:], bot[:, w0*YW:(w0+2)*YW])
                nc.scalar.dma_start(Y_d[w0, i*BT:(i+1)*BT, :], y[:, :YW])
                nc.scalar.dma_start(Y_d[w0 + 1, i*BT:(i+1)*BT, :], y[:, YW:])

    nc.compile()
    return nc


def _tile_x(a):
    """[B, IN_DIM] -> [NBT, 128(p=k%128), NKT*BT] preserving dtype."""
    return np.ascontiguousarray(
        a.reshape(NBT, BT, NKT, 128).transpose(0, 3, 2, 1).reshape(NBT, 128, -1))


def _tile_w(a):
    """[IN_DIM, JPC] -> [NCHUNK*NKT, 128, CHUNK]."""
    return np.ascontiguousarray(
        a.reshape(NKT, 128, NCHUNK, CHUNK).transpose(2, 0, 1, 3).reshape(NSTRIP, 128, CHUNK))


def kernel(x, w_in, b_in, w_in_mask, w_out, b_out, duty_cycle):
    from concourse.bass_utils import run_bass_kernel_spmd
    global LAST_RESULTS

    x = np.ascontiguousarray(x, dtype=np.float32)
    w_in = np.asarray(w_in, dtype=np.float32)
    w_in_mask = np.asarray(w_in_mask, dtype=np.float32)
    w_out = np.asarray(w_out, dtype=np.float32)
    b_in = np.asarray(b_in, dtype=np.float32)
    b_out = np.asarray(b_out, dtype=np.float32)
    duty_cycle = np.asarray(duty_cycle, dtype=np.float32)
    assert x.shape == (B, IN_DIM) and w_in.shape == (ND, IN_DIM)

    has_bin = bool(np.any(b_in))
    has_bout = bool(np.any(b_out))

    key = (has_bin, has_bout)
    if key not in _prog_cache:
        _prog_cache[key] = _build(has_bin, has_bout)
    nc = _prog_cache[key]

    # ---- host-side operand prep: boost folding + f32r rounding + tiling ----
    XRt = _tile_x(_round_f32r(x))                         # [NBT, 128, NKT*BT]

    # w_in[d*OUT + c*UPC + u', k] -> per-core [k, j'=u'*8+d] via reshape/transpose
    Wmask = w_in * w_in_mask                              # [ND, IN_DIM]
    w4 = Wmask.reshape(DPC, NCORES, UPC, IN_DIM)          # [d, c, u', k]
    wof = w_out.reshape(-1)
    boost_all = np.exp((1.0 / DPC - duty_cycle) * BOOST_STRENGTH)  # [DPC, OUT_DIM]

    uprime = np.arange(UPC)
    dd = np.arange(DPC)
    jp_u = np.repeat(uprime, DPC)                         # u'(j') ; j' = u'*8 + d
    jp_d = np.tile(dd, UPC)                               # d(j')

    in_maps = []
    for c in range(NCORES):
        rows = jp_d * OUT_DIM + c * UPC + jp_u            # global w_in row per j'
        bj = boost_all[jp_d, c * UPC + jp_u].astype(np.float32)        # [JPC]
        Wm = np.ascontiguousarray(w4[:, c].transpose(2, 1, 0).reshape(IN_DIM, JPC))
        WRt = _tile_w(_round_f32r(Wm * bj[None, :]))      # boosted weights
        v = jp_d * (OUT_DIM // DPC) + c * (UPC // DPC) + (jp_u // DPC)  # d*256 + c*32 + u'//8
        t = jp_u % DPC
        import ml_dtypes
        wz = (wof[v * ND + v * DPC + t] / bj).astype(ml_dtypes.bfloat16)
        im = {"XR": XRt, "WR": WRt, "WZ": np.ascontiguousarray(wz.reshape(1, JPC))}
        if has_bin:
            im["binb"] = np.ascontiguousarray(
                (b_in[rows] * bj).astype(np.float32).reshape(1, JPC))
        if has_bout:
            # bout4[w*64 + s*8 + q] = b_out[v], v = q*256 + c*32 + 8w + s
            wq = np.arange(NCHUNK * YW)
            wi, si, qi = wq // YW, (wq % YW) // 8, wq % 8
            vv = qi * (OUT_DIM // DPC) + c * (UPC // DPC) + 8 * wi + si
            im["bout"] = np.ascontiguousarray(
                b_out[vv].astype(np.float32).reshape(1, NCHUNK * YW))
        in_maps.append(im)

    import os
    trace = bool(os.environ.get("KERNEL_TRACE"))
    last_err = None
    for _attempt in range(3):
        try:
            res = run_bass_kernel_spmd(nc, in_maps, list(range(NCORES)), trace=trace)
            break
        except Exception as err:  # rare transient device fault on first execute
            last_err = err
            import time as _time
            _time.sleep(2.0)
    else:
        raise last_err
    LAST_RESULTS = res

    # Y4[w, b, s*8+q] (per core) -> y[b, q*256 + c*32 + 8w + s]
    Yc = np.stack([res.results[c]["Y"] for c in range(NCORES)], axis=0)  # [8, NCHUNK, B, 64]
    Yc = Yc.reshape(NCORES, NCHUNK, B, 8, 8)             # [c, w, b, s, q]
    y = Yc.transpose(2, 4, 0, 1, 3).reshape(B, OUT_DIM)  # [b, q, c, w, s] -> v = q*256+c*32+8w+s
    return np.ascontiguousarray(y)
